# revision 16
# baseline (speedup 1.0000x reference)
"""MoE gate (softmax + top-8 routing + seq-aux loss) for 8 Trainium2 cores.

Strategy (data/sequence parallel):
  - 32768 tokens are sharded 4096/core across 8 NeuronCores; the tiny gate
    weight is replicated.
  - Each core computes logits = x @ W^T via PE matmul.  The contraction dim
    (hidden dim d) must sit on SBUF partitions, so the host stages each
    token-shard in d-major ("transposed") layout; x-tiles then load with
    fully contiguous DMA descriptors and serve directly as the stationary
    matmul operand, producing scores in [token, expert] layout with zero
    on-device transposes.
  - Per 128-token block: ACT computes exp(logits) fused with the row-sum Z;
    DVE max8/max_index give the top-8 values+indices in one pass each;
    top-8 weights are normalized with a reciprocal+scale.
  - The seq-aux loss needs column sums of softmax probs: sum_t exp(l)/Z.
    That is a rank-1 matmul with r = 1/Z as the stationary operand,
    accumulated in PSUM across the whole shard (PE divides implicitly).
  - Host combines: concatenates top-k outputs, computes the expert histogram
    (ce) exactly from the returned indices, and folds the per-core prob-sum
    partials into the scalar aux loss.

Built on bacc.Bacc (not raw bass.Bass): the bacc compile pass splits
multi-semaphore waits into EventSemaphore instructions, which the TRN2 ISA
requires (every other instruction has a single sync-wait slot).
"""

import sys

if "/opt/trn_rl_repo" not in sys.path:
    sys.path.insert(0, "/opt/trn_rl_repo")

import numpy as np

import concourse.bacc as bacc
import concourse.mybir as mybir
from concourse.bass_utils import run_bass_kernel_spmd
from concourse.tile import TileContext

N_CORES = 8
T_TOTAL = 32768          # 4 * 8192 tokens
DIM = 2048
E = 64                   # experts
K = 8                    # top-k
TPC = T_TOTAL // N_CORES  # tokens per core = 4096
BSZ = 4
SEQ = 8192
ALPHA = 0.1
P = 128                  # SBUF partitions
NCHUNK = DIM // P        # 16 contraction chunks
STRIP = 512              # tokens per DMA strip
NSTRIP = TPC // STRIP    # 8
NSUB = STRIP // P        # 4 blocks of 128 tokens per strip
NBLK = TPC // P          # 32 blocks of 128 tokens per core

_f32 = mybir.dt.float32
_u32 = mybir.dt.uint32


def _build_bass():
    nc = bacc.Bacc("TRN2")

    xT = nc.dram_tensor("xT", [DIM, TPC], _f32, kind="ExternalInput")
    wT = nc.dram_tensor("wT", [DIM, E], _f32, kind="ExternalInput")
    # Staged outputs: column block b holds the 8 values for tokens
    # [b*128, (b+1)*128), token t = b*128 + partition.  Host de-interleaves.
    out_w = nc.dram_tensor("out_w", [P, NBLK * K], _f32, kind="ExternalOutput")
    out_i = nc.dram_tensor("out_i", [P, NBLK * K], _u32, kind="ExternalOutput")
    # sum_t softmax_probs[t, e] over this core's tokens.
    out_aux = nc.dram_tensor("out_aux", [1, E], _f32, kind="ExternalOutput")

    with TileContext(nc) as tc:
        with (
            tc.tile_pool(name="singles", bufs=1) as singles,
            tc.tile_pool(name="xstrip", bufs=2) as xpool,
            tc.tile_pool(name="work", bufs=6) as work,
            tc.tile_pool(name="small", bufs=6) as small,
            tc.tile_pool(name="psum", bufs=7, space="PSUM") as psum,
            tc.tile_pool(name="psum_aux", bufs=1, space="PSUM") as psum_aux,
        ):
            # Replicated gate weight, chunked [p, chunk, expert].
            wt_sb = singles.tile([P, NCHUNK, E], _f32)
            nc.sync.dma_start(
                out=wt_sb, in_=wT.rearrange("(c p) e -> p c e", p=P)
            )

            # Output staging (written piecewise, DMA'd once at the end).
            ow_sb = singles.tile([P, NBLK * K], _f32)
            oi_sb = singles.tile([P, NBLK * K], _u32)

            aux_ps = psum_aux.tile([1, E], _f32)

            for s in range(NSTRIP):
                xt = xpool.tile([P, NCHUNK, STRIP], _f32)
                nc.sync.dma_start(
                    out=xt,
                    in_=xT[:, s * STRIP : (s + 1) * STRIP].rearrange(
                        "(c p) t -> p c t", p=P
                    ),
                )
                for u in range(NSUB):
                    blk = s * NSUB + u
                    ps = psum.tile([P, E], _f32, tag="ps")
                    for c in range(NCHUNK):
                        nc.tensor.matmul(
                            ps,
                            lhsT=xt[:, c, u * P : (u + 1) * P],
                            rhs=wt_sb[:, c, :],
                            start=(c == 0),
                            stop=(c == NCHUNK - 1),
                        )
                    # exps = exp(logits); zs[:,1] = Z = sum_e exps  (fused)
                    exps = work.tile([P, E], _f32, tag="exps")
                    zs = small.tile([P, 2], _f32, tag="zs")
                    rec = small.tile([P, 2], _f32, tag="rec")
                    top8 = small.tile([P, K], _f32, tag="top8")
                    nc.scalar.activation(
                        exps,
                        ps,
                        mybir.ActivationFunctionType.Exp,
                        accum_out=zs[:, 1:2],
                    )
                    nc.vector.max(out=top8, in_=exps)
                    nc.vector.max_index(
                        out=oi_sb[:, blk * K : (blk + 1) * K],
                        in_max=top8,
                        in_values=exps,
                    )
                    # zs[:,0] = sum of top-8 exps
                    nc.vector.tensor_reduce(
                        zs[:, 0:1],
                        top8,
                        axis=mybir.AxisListType.X,
                        op=mybir.AluOpType.add,
                    )
                    nc.vector.reciprocal(rec, zs)
                    # normalized top-k weights
                    nc.vector.tensor_scalar_mul(
                        ow_sb[:, blk * K : (blk + 1) * K], top8, rec[:, 0:1]
                    )
                    # aux partial: out_aux[e] += sum_t exps[t, e] / Z[t]
                    nc.tensor.matmul(
                        aux_ps,
                        lhsT=rec[:, 1:2],
                        rhs=exps,
                        start=(blk == 0),
                        stop=(blk == NBLK - 1),
                        skip_group_check=True,
                    )

            aux_sb = singles.tile([1, E], _f32)
            nc.vector.tensor_copy(aux_sb, aux_ps)
            nc.sync.dma_start(out=out_aux[:, :], in_=aux_sb)
            nc.sync.dma_start(out=out_w[:, :], in_=ow_sb)
            nc.sync.dma_start(out=out_i[:, :], in_=oi_sb)

    # Run the bacc compile pass (splits multi-semaphore waits etc.) NOW —
    # the PJRT/axon execute path serializes with to_json_bytes() and would
    # otherwise ship un-lowered BIR to walrus.
    nc.finalize()
    return nc


_NC_CACHE = None


def _get_nc():
    global _NC_CACHE
    if _NC_CACHE is None:
        _NC_CACHE = _build_bass()
    return _NC_CACHE


def _make_in_maps(hidden_states: np.ndarray, weight: np.ndarray):
    x2 = np.ascontiguousarray(np.asarray(hidden_states, np.float32)).reshape(
        T_TOTAL, DIM
    )
    wt = np.ascontiguousarray(np.asarray(weight, np.float32).T)  # [DIM, E]
    in_maps = []
    for c in range(N_CORES):
        shard = x2[c * TPC : (c + 1) * TPC]  # [TPC, DIM]
        in_maps.append({"xT": np.ascontiguousarray(shard.T), "wT": wt})
    return in_maps


def _combine(results):
    w_parts, i_parts, p_parts = [], [], []
    for r in results:
        # [P, NBLK, K] -> token t = blk*128 + p
        w_parts.append(
            r["out_w"].reshape(P, NBLK, K).transpose(1, 0, 2).reshape(TPC, K)
        )
        i_parts.append(
            r["out_i"]
            .reshape(P, NBLK, K)
            .transpose(1, 0, 2)
            .reshape(TPC, K)
            .astype(np.int32)
        )
        p_parts.append(r["out_aux"].reshape(E).astype(np.float64))

    topk_weight = np.concatenate(w_parts, axis=0)  # [T_TOTAL, K]
    topk_idx = np.concatenate(i_parts, axis=0)     # [T_TOTAL, K] int32

    # seq-aux loss: ce from the exact index histogram; mean_scores from the
    # per-core prob-sum partials.
    idx_b = topk_idx.reshape(BSZ, SEQ * K)
    ce = np.stack(
        [np.bincount(idx_b[b], minlength=E) for b in range(BSZ)]
    ).astype(np.float64)
    ce *= E / (SEQ * K)
    cores_per_batch = N_CORES // BSZ  # 2
    mean_scores = np.stack(
        [
            sum(p_parts[b * cores_per_batch + j] for j in range(cores_per_batch))
            / SEQ
            for b in range(BSZ)
        ]
    )  # [BSZ, E]
    aux_loss = np.float32((ce * mean_scores).sum(axis=1).mean() * ALPHA)
    return topk_idx, topk_weight, aux_loss


def kernel(hidden_states: np.ndarray, weight: np.ndarray):
    nc = _get_nc()
    in_maps = _make_in_maps(hidden_states, weight)
    res = run_bass_kernel_spmd(nc, in_maps, core_ids=list(range(N_CORES)))
    return _combine(res.results)


if __name__ == "__main__":
    rng = np.random.default_rng(0)
    h = rng.standard_normal((BSZ, SEQ, DIM), dtype=np.float32)
    w = (rng.standard_normal((E, DIM), dtype=np.float32) * 0.0127).astype(
        np.float32
    )
    idx, wts, aux = kernel(h, w)
    print(idx.shape, idx.dtype, wts.shape, wts.dtype, aux)
    print("row0 idx:", idx[0], "w:", wts[0], "sum:", wts[0].sum())


# revision 18
# speedup vs baseline: 1.1648x; 1.1648x over previous
"""MoE gate (softmax + top-8 routing + seq-aux loss) for 8 Trainium2 cores.

Strategy (data/sequence parallel):
  - 32768 tokens are sharded 4096/core across 8 NeuronCores; the tiny gate
    weight is replicated.
  - Each core computes logits = x @ W^T via PE matmul.  The contraction dim
    (hidden dim d) must sit on SBUF partitions, so the host stages each
    token-shard in d-major ("transposed") layout; x-tiles then load with
    fully contiguous DMA descriptors and serve directly as the stationary
    matmul operand, producing scores in [token, expert] layout with zero
    on-device transposes.
  - Per 128-token block: ACT computes exp(logits) fused with the row-sum Z;
    DVE max8/max_index give the top-8 values+indices in one pass each;
    top-8 weights are normalized with a reciprocal+scale.
  - The seq-aux loss needs column sums of softmax probs: sum_t exp(l)/Z.
    That is a rank-1 matmul with r = 1/Z as the stationary operand,
    accumulated in PSUM across the whole shard (PE divides implicitly).
  - Host combines: concatenates top-k outputs, computes the expert histogram
    (ce) exactly from the returned indices, and folds the per-core prob-sum
    partials into the scalar aux loss.

Built on bacc.Bacc (not raw bass.Bass): the bacc compile pass splits
multi-semaphore waits into EventSemaphore instructions, which the TRN2 ISA
requires (every other instruction has a single sync-wait slot).
"""

import sys

if "/opt/trn_rl_repo" not in sys.path:
    sys.path.insert(0, "/opt/trn_rl_repo")

import numpy as np

import concourse.bacc as bacc
import concourse.mybir as mybir
from concourse.bass_utils import run_bass_kernel_spmd
from concourse.tile import TileContext

N_CORES = 8
T_TOTAL = 32768          # 4 * 8192 tokens
DIM = 2048
E = 64                   # experts
K = 8                    # top-k
TPC = T_TOTAL // N_CORES  # tokens per core = 4096
BSZ = 4
SEQ = 8192
ALPHA = 0.1
P = 128                  # SBUF partitions
NCHUNK = DIM // P        # 16 contraction chunks
STRIP = 256              # tokens per DMA strip
NSTRIP = TPC // STRIP    # 8
NSUB = STRIP // P        # 4 blocks of 128 tokens per strip
NBLK = TPC // P          # 32 blocks of 128 tokens per core

_f32 = mybir.dt.float32
_u32 = mybir.dt.uint32


def _build_bass(strip=STRIP, xbufs=4, psbufs=7, wbufs=6):
    nstrip = TPC // strip
    nsub = strip // P
    nc = bacc.Bacc("TRN2")

    # Strip-major, partition-contiguous staging layout: each strip DMA reads
    # 128 fully contiguous rows (chunk-all-tokens per partition).
    xT = nc.dram_tensor("xT", [nstrip, P, NCHUNK, strip], _f32, kind="ExternalInput")
    wT = nc.dram_tensor("wT", [DIM, E], _f32, kind="ExternalInput")
    # Staged outputs: column block b holds the 8 values for tokens
    # [b*128, (b+1)*128), token t = b*128 + partition.  Host de-interleaves.
    out_w = nc.dram_tensor("out_w", [P, NBLK * K], _f32, kind="ExternalOutput")
    out_i = nc.dram_tensor("out_i", [P, NBLK * K], _u32, kind="ExternalOutput")
    # sum_t softmax_probs[t, e] over this core's tokens.
    out_aux = nc.dram_tensor("out_aux", [1, E], _f32, kind="ExternalOutput")

    with TileContext(nc) as tc:
        with (
            tc.tile_pool(name="singles", bufs=1) as singles,
            tc.tile_pool(name="xstrip", bufs=xbufs) as xpool,
            tc.tile_pool(name="work", bufs=wbufs) as work,
            tc.tile_pool(name="small", bufs=wbufs) as small,
            tc.tile_pool(name="psum", bufs=psbufs, space="PSUM") as psum,
            tc.tile_pool(name="psum_aux", bufs=1, space="PSUM") as psum_aux,
        ):
            # Replicated gate weight, chunked [p, chunk, expert].
            wt_sb = singles.tile([P, NCHUNK, E], _f32)
            nc.sync.dma_start(
                out=wt_sb, in_=wT.rearrange("(c p) e -> p c e", p=P)
            )

            # Output staging (written piecewise, DMA'd once at the end).
            ow_sb = singles.tile([P, NBLK * K], _f32)
            oi_sb = singles.tile([P, NBLK * K], _u32)

            aux_ps = psum_aux.tile([1, E], _f32)

            for s in range(nstrip):
                xt = xpool.tile([P, NCHUNK, strip], _f32)
                nc.sync.dma_start(out=xt, in_=xT[s])
                for u in range(nsub):
                    blk = s * nsub + u
                    ps = psum.tile([P, E], _f32, tag="ps")
                    for c in range(NCHUNK):
                        nc.tensor.matmul(
                            ps,
                            lhsT=xt[:, c, u * P : (u + 1) * P],
                            rhs=wt_sb[:, c, :],
                            start=(c == 0),
                            stop=(c == NCHUNK - 1),
                        )
                    # exps = exp(logits); zs[:,1] = Z = sum_e exps  (fused)
                    exps = work.tile([P, E], _f32, tag="exps")
                    zs = small.tile([P, 2], _f32, tag="zs")
                    rec = small.tile([P, 2], _f32, tag="rec")
                    top8 = small.tile([P, K], _f32, tag="top8")
                    nc.scalar.activation(
                        exps,
                        ps,
                        mybir.ActivationFunctionType.Exp,
                        accum_out=zs[:, 1:2],
                    )
                    nc.vector.max(out=top8, in_=exps)
                    nc.vector.max_index(
                        out=oi_sb[:, blk * K : (blk + 1) * K],
                        in_max=top8,
                        in_values=exps,
                    )
                    # zs[:,0] = sum of top-8 exps
                    nc.vector.tensor_reduce(
                        zs[:, 0:1],
                        top8,
                        axis=mybir.AxisListType.X,
                        op=mybir.AluOpType.add,
                    )
                    nc.vector.reciprocal(rec, zs)
                    # normalized top-k weights
                    nc.vector.tensor_scalar_mul(
                        ow_sb[:, blk * K : (blk + 1) * K], top8, rec[:, 0:1]
                    )
                    # aux partial: out_aux[e] += sum_t exps[t, e] / Z[t]
                    nc.tensor.matmul(
                        aux_ps,
                        lhsT=rec[:, 1:2],
                        rhs=exps,
                        start=(blk == 0),
                        stop=(blk == NBLK - 1),
                        skip_group_check=True,
                    )

            aux_sb = singles.tile([1, E], _f32)
            nc.vector.tensor_copy(aux_sb, aux_ps)
            nc.sync.dma_start(out=out_aux[:, :], in_=aux_sb)
            nc.sync.dma_start(out=out_w[:, :], in_=ow_sb)
            nc.sync.dma_start(out=out_i[:, :], in_=oi_sb)

    # Run the bacc compile pass (splits multi-semaphore waits etc.) NOW —
    # the PJRT/axon execute path serializes with to_json_bytes() and would
    # otherwise ship un-lowered BIR to walrus.
    nc.finalize()
    return nc


_NC_CACHE = None


def _get_nc():
    global _NC_CACHE
    if _NC_CACHE is None:
        _NC_CACHE = _build_bass()
    return _NC_CACHE


def _make_in_maps(hidden_states: np.ndarray, weight: np.ndarray, strip=STRIP):
    nstrip = TPC // strip
    x2 = np.asarray(hidden_states, np.float32).reshape(T_TOTAL, DIM)
    wt = np.ascontiguousarray(np.asarray(weight, np.float32).T)  # [DIM, E]
    in_maps = []
    for c in range(N_CORES):
        shard = x2[c * TPC : (c + 1) * TPC]  # [TPC, DIM]
        # [nstrip, P, NCHUNK, strip]: xd[s, p, ch, t] = shard[s*strip+t, ch*128+p]
        xd = np.ascontiguousarray(
            shard.reshape(nstrip, strip, NCHUNK, P).transpose(0, 3, 2, 1)
        )
        in_maps.append({"xT": xd, "wT": wt})
    return in_maps


def _combine(results):
    w_parts, i_parts, p_parts = [], [], []
    for r in results:
        # [P, NBLK, K] -> token t = blk*128 + p
        w_parts.append(
            r["out_w"].reshape(P, NBLK, K).transpose(1, 0, 2).reshape(TPC, K)
        )
        i_parts.append(
            r["out_i"]
            .reshape(P, NBLK, K)
            .transpose(1, 0, 2)
            .reshape(TPC, K)
            .astype(np.int32)
        )
        p_parts.append(r["out_aux"].reshape(E).astype(np.float64))

    topk_weight = np.concatenate(w_parts, axis=0)  # [T_TOTAL, K]
    topk_idx = np.concatenate(i_parts, axis=0)     # [T_TOTAL, K] int32

    # seq-aux loss: ce from the exact index histogram; mean_scores from the
    # per-core prob-sum partials.
    idx_b = topk_idx.reshape(BSZ, SEQ * K)
    ce = np.stack(
        [np.bincount(idx_b[b], minlength=E) for b in range(BSZ)]
    ).astype(np.float64)
    ce *= E / (SEQ * K)
    cores_per_batch = N_CORES // BSZ  # 2
    mean_scores = np.stack(
        [
            sum(p_parts[b * cores_per_batch + j] for j in range(cores_per_batch))
            / SEQ
            for b in range(BSZ)
        ]
    )  # [BSZ, E]
    aux_loss = np.float32((ce * mean_scores).sum(axis=1).mean() * ALPHA)
    return topk_idx, topk_weight, aux_loss


def kernel(hidden_states: np.ndarray, weight: np.ndarray):
    nc = _get_nc()
    in_maps = _make_in_maps(hidden_states, weight)
    res = run_bass_kernel_spmd(nc, in_maps, core_ids=list(range(N_CORES)))
    return _combine(res.results)


if __name__ == "__main__":
    rng = np.random.default_rng(0)
    h = rng.standard_normal((BSZ, SEQ, DIM), dtype=np.float32)
    w = (rng.standard_normal((E, DIM), dtype=np.float32) * 0.0127).astype(
        np.float32
    )
    idx, wts, aux = kernel(h, w)
    print(idx.shape, idx.dtype, wts.shape, wts.dtype, aux)
    print("row0 idx:", idx[0], "w:", wts[0], "sum:", wts[0].sum())
